# revision 20
# baseline (speedup 1.0000x reference)
"""Trainium2 Bass kernel for LoRA attention prefill (B=4, S=1024, D=4096, H=32).

Sharding: tensor-parallel over heads. Each of the 8 cores computes 4 heads
(512 of the 4096 q/k/v features, column-shard of wq/wk/wv) and a row-shard
of wo, producing a full-shape [T, D] partial output; partials are summed on
the host.

Design notes:
  - LoRA folded into wq/wv on the host (exact algebra) - no device LoRA work.
  - Causal masking is multiplicative (0/1 bf16 after exp); diagonal score
    blocks only compute the live query range (partial-width matmuls).
  - Softmax denominators: exp tiles accumulated on DVE into a bf16 SBUF acc;
    one ones-matrix matmul per head broadcasts the denominator to all 128
    partitions; reciprocal_approx_fast (DVE) replaces the slow serial
    RECIPROCAL.
  - The PE instruction stream is software-pipelined end to end: attention
    rounds (which are exp/Scalar-latency bound) are interleaved with stage-A
    projection and stage-C wo matmul chunks via generators, so the in-order
    PE queue always has independent work. PV matmuls lag two rounds behind
    their score matmuls.
  - Startup weight DMAs are split into consumption-order chunks; the next
    batch's first x tile is prefetched before stage C.
"""
import sys
from contextlib import ExitStack

sys.path.insert(0, "/opt/trn_rl_repo")

import numpy as np
import ml_dtypes

import concourse.bass as bass
import concourse.mybir as mybir
import concourse.tile as tile
from concourse import bacc
from concourse.bass_utils import run_bass_kernel_spmd
from concourse.tile import TileContext

B, S, D = 4, 1024, 4096
H, HD = 32, 128
R = 16
LORA_SCALE = 2.0
N_CORES = 8
HPC = H // N_CORES            # heads per core
FPC = HPC * HD                # features per core = 512
T = B * S                     # 4096 tokens
TT = 256                      # stage-A T-tile (tokens)
NTT = S // TT                 # T-tiles per batch = 4
SCALE = float(1.0 / np.sqrt(HD))
BF = mybir.dt.bfloat16
F32 = mybir.dt.float32


def _bf(a):
    return np.ascontiguousarray(np.asarray(a, np.float32).astype(ml_dtypes.bfloat16))


def _core_perm(c):
    hs = [HPC * c + i for i in range(HPC)]
    ev = np.arange(0, HD, 2)
    od = np.arange(1, HD, 2)
    out = []
    for pair in (0, 1):
        h0, h1 = hs[2 * pair], hs[2 * pair + 1]
        out.append(h0 * HD + ev)
        out.append(h1 * HD + ev)
        out.append(h0 * HD + od)
        out.append(h1 * HD + od)
    return np.concatenate(out)


def _check_causal(mask):
    iu = np.triu_indices(S, k=1)
    il = np.tril_indices(S, k=0)
    return bool((mask[iu] <= -1e8).all() and (mask[il] == 0).all())


def _host_prep(x, wq_w, wq_a, wq_b, wk_w, wv_w, wv_a, wv_b, wo_w,
               freqs_cos, freqs_sin, mask):
    x2 = np.asarray(x, np.float32).reshape(T, D)
    xT = _bf(x2.T)

    # fold LoRA into the dense weights: y = x(W + s*B@A)^T exactly
    wq_eff = np.asarray(wq_w, np.float32) + LORA_SCALE * (
        np.asarray(wq_b, np.float32) @ np.asarray(wq_a, np.float32))
    wv_eff = np.asarray(wv_w, np.float32) + LORA_SCALE * (
        np.asarray(wv_b, np.float32) @ np.asarray(wv_a, np.float32))
    wk = np.asarray(wk_w, np.float32)

    cosT = np.asarray(freqs_cos, np.float32).T
    sinT = np.asarray(freqs_sin, np.float32).T
    cc = np.ascontiguousarray(np.tile(cosT, (2, B)).astype(np.float32))
    ss = np.ascontiguousarray(np.tile(sinT, (2, B)).astype(np.float32))

    mask = np.asarray(mask, np.float32)
    causal = _check_causal(mask)
    if causal:
        # 0/1 multiplicative triangle for the 128x128 diagonal strips
        tri = np.tril(np.ones((128, 128), np.float32)).T  # [k,q]: 1 if k<=q
        maskp = _bf(np.broadcast_to(tri, (4, 128, 128)))
    else:
        mT = mask.T * np.float32(np.sqrt(HD))
        maskp = np.zeros((8, 128, 2, 512), np.float32)
        for qh in range(2):
            for j in range(8):
                maskp[j, :, qh, :] = mT[j * 128:(j + 1) * 128,
                                        qh * 512:(qh + 1) * 512]

    shared = dict(xT=xT, cc=cc, ss=ss, maskp=maskp)
    cores = []
    for c in range(N_CORES):
        perm = _core_perm(c)
        sl = slice(c * FPC, (c + 1) * FPC)
        cores.append(dict(
            wqT=_bf(wq_eff[perm, :].T),
            wkT=_bf(wk[perm, :].T),
            wvT=_bf(wv_eff[sl, :].T),
            woT=_bf(np.asarray(wo_w, np.float32)[:, sl].T),
        ))
    return shared, cores, causal


def _zip_chunks(*gens, lead=0):
    """Round-robin drive generators to completion. `lead` pulls that many
    chunks from the first generator before starting the round-robin."""
    gens = list(gens)
    for _ in range(lead):
        try:
            next(gens[0])
        except StopIteration:
            gens.pop(0)
            break
    while gens:
        for g in list(gens):
            try:
                next(g)
            except StopIteration:
                gens.remove(g)


def _build_program(causal):
    nc = bacc.Bacc("TRN2", num_devices=N_CORES)

    xT = nc.dram_tensor("xT", [D, T], BF, kind="ExternalInput").ap()
    wqT = nc.dram_tensor("wqT", [D, FPC], BF, kind="ExternalInput").ap()
    wkT = nc.dram_tensor("wkT", [D, FPC], BF, kind="ExternalInput").ap()
    wvT = nc.dram_tensor("wvT", [D, FPC], BF, kind="ExternalInput").ap()
    woT = nc.dram_tensor("woT", [FPC, D], BF, kind="ExternalInput").ap()
    cc = nc.dram_tensor("cc", [128, T], F32, kind="ExternalInput").ap()
    ss = nc.dram_tensor("ss", [128, T], F32, kind="ExternalInput").ap()
    if causal:
        maskp = nc.dram_tensor("maskp", [4, 128, 128], BF,
                               kind="ExternalInput").ap()
    else:
        maskp = nc.dram_tensor("maskp", [8, 128, 2, 512], F32,
                               kind="ExternalInput").ap()
    y = nc.dram_tensor("y", [T, D], F32, kind="ExternalOutput").ap()

    with TileContext(nc) as tc, ExitStack() as ctx:
        wpool = ctx.enter_context(tc.tile_pool(name="wpool", bufs=1))
        xpool = ctx.enter_context(tc.tile_pool(name="xpool", bufs=2))
        ccp = ctx.enter_context(tc.tile_pool(name="ccp", bufs=4))
        qkvp = ctx.enter_context(tc.tile_pool(name="qkvp", bufs=1))
        expp = ctx.enter_context(tc.tile_pool(name="expp", bufs=9))
        accp = ctx.enter_context(tc.tile_pool(name="accp", bufs=3))
        recp = ctx.enter_context(tc.tile_pool(name="recp", bufs=2))
        otp = ctx.enter_context(tc.tile_pool(name="otp", bufs=1))
        outp = ctx.enter_context(tc.tile_pool(name="outp", bufs=3))
        tmpp = ctx.enter_context(tc.tile_pool(name="tmpp", bufs=4))
        stp = ctx.enter_context(tc.tile_pool(name="stp", bufs=4))
        wop = ctx.enter_context(tc.tile_pool(name="wop", bufs=2))
        psA = ctx.enter_context(tc.tile_pool(name="psA", bufs=3, space="PSUM"))
        psOT = ctx.enter_context(tc.tile_pool(name="psOT", bufs=2,
                                              space="PSUM"))
        psSC = ctx.enter_context(tc.tile_pool(name="psSC", bufs=3,
                                              space="PSUM"))

        xre = xT.rearrange("(o p) t -> p o t", p=128)
        # prefetched stage-A input tiles, keyed by (b, tt)
        fetched = {}

        def fetch_x(b, tt, xq=None):
            t0 = b * S + tt * TT
            x_sb = xpool.tile([128, 32, TT], BF, tag="x", name="x_sb")
            (xq or nc.sync).dma_start(x_sb[:], xre[:, :, t0:t0 + TT])
            cc_sb = ccp.tile([128, TT], F32, tag="cc", name="cc_sb")
            nc.sync.dma_start(cc_sb[:], cc[:, t0:t0 + TT])
            ss_sb = ccp.tile([128, TT], F32, tag="ss", name="ss_sb")
            nc.sync.dma_start(ss_sb[:], ss[:, t0:t0 + TT])
            fetched[(b, tt)] = (x_sb, cc_sb, ss_sb)

        # first input tile before the weights so compute starts early
        fetch_x(0, 0)

        # resident weights, split into consumption-order chunks so the first
        # projection matmuls can start before the full weight set has landed
        wq_sb = wpool.tile([128, 32, FPC], BF, tag="wq")
        wk_sb = wpool.tile([128, 32, FPC], BF, tag="wk")
        wv_sb = wpool.tile([128, 32, FPC], BF, tag="wv")
        wqr = wqT.rearrange("(o p) f -> p o f", p=128)
        wkr = wkT.rearrange("(o p) f -> p o f", p=128)
        wvr = wvT.rearrange("(o p) f -> p o f", p=128)
        # spread the one-time weight loads over all three DMA lanes in
        # first-use order: sync also carries the x tiles, scalar must be
        # clear before tt0's RoPE regroup copies, gpsimd is the slow bulk
        # lane for whatever has the latest deadline
        nc.scalar.dma_start(wq_sb[:, :, 128:256], wqr[:, :, 128:256])
        nc.scalar.dma_start(wk_sb[:, :, 128:256], wkr[:, :, 128:256])
        nc.sync.dma_start(wq_sb[:, :, 0:128], wqr[:, :, 0:128])
        nc.sync.dma_start(wk_sb[:, :, 0:128], wkr[:, :, 0:128])
        nc.gpsimd.dma_start(wq_sb[:, :, 256:384], wqr[:, :, 256:384])
        nc.gpsimd.dma_start(wq_sb[:, :, 384:512], wqr[:, :, 384:512])
        nc.gpsimd.dma_start(wk_sb[:, :, 256:384], wkr[:, :, 256:384])
        nc.gpsimd.dma_start(wk_sb[:, :, 384:512], wkr[:, :, 384:512])
        nc.sync.dma_start(wv_sb[:, :, 0:256], wvr[:, :, 0:256])
        nc.gpsimd.dma_start(wv_sb[:, :, 256:512], wvr[:, :, 256:512])
        if causal:
            mask_sb = wpool.tile([128, 4, 128], BF, tag="mask")
            nc.gpsimd.dma_start(mask_sb[:], maskp.rearrange("j p n -> p j n"))
        else:
            mask_sb = wpool.tile([128, 8, 2, 512], F32, tag="mask")
            nc.sync.dma_start(mask_sb[:],
                              maskp.rearrange("j p q n -> p j q n"))
        ones_m = wpool.tile([128, 128], BF, tag="onesm")
        nc.gpsimd.memset(ones_m[:], 1.0)
        fetch_x(0, 1)

        pending_c_tail = None
        for b in range(B):
            Q_sb = qkvp.tile([128, 4, S], BF, tag="Qsb")
            K_sb = qkvp.tile([128, 4, S], BF, tag="Ksb")
            V_sb = qkvp.tile([128, 8, FPC], BF, tag="Vsb")
            OT_sb = otp.tile([128, 4, S], BF, tag="OT")

            def stage_a(tt, prefetch_next):
                """Generator: QK pair groups + V blocks for one t-tile."""
                x_sb, cc_sb, ss_sb = fetched.pop((b, tt))
                if prefetch_next is not None:
                    fetch_x(*prefetch_next)
                toff = tt * TT
                for dst_sb, w_sb in ((Q_sb, wq_sb), (K_sb, wk_sb)):
                    for pair in range(2):
                        # one PSUM bank holds both the real and imag halves
                        # (the r accumulation group completes before the i
                        # group's start clears the bank's has_written bits)
                        ps = psA.tile([128, 512], F32, tag="psa",
                                      name="ps_qk")
                        for ri in range(2):
                            f0 = pair * 256 + ri * 128
                            for d in range(32):
                                nc.tensor.matmul(
                                    ps[:, ri * TT:ri * TT + TT],
                                    w_sb[:, d, f0:f0 + 128],
                                    x_sb[:, d, :], start=(d == 0),
                                    stop=(d == 31))
                        ps_r = ps[:, 0:TT]
                        ps_i = ps[:, TT:2 * TT]
                        t1 = tmpp.tile([128, TT], F32, tag="t", name="t1")
                        nc.vector.tensor_mul(t1[:], ps_r, cc_sb[:])
                        t2 = tmpp.tile([128, TT], F32, tag="t", name="t2")
                        nc.vector.tensor_mul(t2[:], ps_i, ss_sb[:])
                        st_r = stp.tile([128, TT], BF, tag="st", name="st_r")
                        nc.vector.tensor_tensor(
                            st_r[:], t1[:], t2[:], mybir.AluOpType.subtract)
                        t3 = tmpp.tile([128, TT], F32, tag="t", name="t3")
                        nc.vector.tensor_mul(t3[:], ps_r, ss_sb[:])
                        t4 = tmpp.tile([128, TT], F32, tag="t", name="t4")
                        nc.vector.tensor_mul(t4[:], ps_i, cc_sb[:])
                        st_i = stp.tile([128, TT], BF, tag="st", name="st_i")
                        nc.vector.tensor_tensor(
                            st_i[:], t3[:], t4[:], mybir.AluOpType.add)
                        # shuffle into head-contiguous blocks: head h of this
                        # pair = [r half; i half] on partitions [0:64|64:128]
                        for hh in range(2):
                            h_loc = 2 * pair + hh
                            nc.scalar.dma_start(
                                dst_sb[0:64, h_loc, toff:toff + TT],
                                st_r[hh * 64:(hh + 1) * 64, :])
                            nc.scalar.dma_start(
                                dst_sb[64:128, h_loc, toff:toff + TT],
                                st_i[hh * 64:(hh + 1) * 64, :])
                        yield
                for v in range(TT // 128):
                    tb = tt * (TT // 128) + v
                    ps_v = psA.tile([128, 512], F32, tag="psa", name="ps_v")
                    for d in range(32):
                        nc.tensor.matmul(
                            ps_v[:], x_sb[:, d, v * 128:(v + 1) * 128],
                            wv_sb[:, d, :], start=(d == 0), stop=(d == 31))
                    nc.scalar.copy(V_sb[:, tb, :], ps_v[:])
                    yield

            def attn_half(qh):
                """Generator: attention rounds for one query half."""
                q0 = qh * 512
                kbs = list(range(0, qh * 4 + 4)) if causal else list(range(8))
                nkb = len(kbs)
                LAG = 3

                def width(kb):
                    if causal and kb - 4 * qh >= 0:
                        return (kb - 4 * qh) * 128
                    return 0

                for hp in range(2):
                    heads = (2 * hp, 2 * hp + 1)
                    ps_ot = {}
                    acc = {}
                    e_t = {}
                    for l in heads:
                        ps_ot[l] = psOT.tile([128, 512], F32, tag="psot",
                                             name="ps_ot")
                        acc[l] = accp.tile([128, 512], BF, tag="acc",
                                           name="acc")

                    def emit_sc(l, j):
                        kb = kbs[j]
                        qlo = width(kb)
                        ps_sc = psSC.tile([128, 512], F32, tag="pssc",
                                          name="ps_sc")
                        nc.tensor.matmul(
                            ps_sc[:, qlo:512],
                            K_sb[:, l, kb * 128:kb * 128 + 128],
                            Q_sb[:, l, q0 + qlo:q0 + 512],
                            start=True, stop=True)
                        e_sb = expp.tile([128, 512], BF, tag="e", name="e_sb")
                        if causal:
                            nc.scalar.activation(
                                e_sb[:, qlo:512], ps_sc[:, qlo:512],
                                mybir.ActivationFunctionType.Exp, scale=SCALE)
                            jj = kb - 4 * qh
                            if jj >= 0:
                                nc.vector.tensor_mul(
                                    e_sb[:, qlo:qlo + 128],
                                    e_sb[:, qlo:qlo + 128],
                                    mask_sb[:, jj, :])
                        else:
                            nc.vector.tensor_add(
                                ps_sc[:], ps_sc[:], mask_sb[:, kb, qh, :])
                            nc.scalar.activation(
                                e_sb[:], ps_sc[:],
                                mybir.ActivationFunctionType.Exp, scale=SCALE)
                        e_t[(l, j)] = e_sb

                    def emit_pv(l, j):
                        kb = kbs[j]
                        qlo = width(kb)
                        e_sb = e_t.pop((l, j))
                        nc.tensor.matmul(
                            ps_ot[l][:, qlo:512],
                            V_sb[:, kb, l * 128:(l + 1) * 128],
                            e_sb[:, qlo:512],
                            start=(j == 0), stop=(j == nkb - 1))
                        if j == 0:
                            nc.vector.tensor_copy(acc[l][:], e_sb[:])
                        else:
                            nc.vector.tensor_add(
                                acc[l][:, qlo:512], acc[l][:, qlo:512],
                                e_sb[:, qlo:512])

                    # software pipeline: PV lags LAG key blocks behind scores
                    for j in range(nkb):
                        for l in heads:
                            emit_sc(l, j)
                            if j >= LAG:
                                emit_pv(l, j - LAG)
                        yield
                    for j in range(max(nkb - LAG, 0), nkb):
                        for l in heads:
                            emit_pv(l, j)
                    # normalization tail for this head pair
                    rec = {}
                    for l in heads:
                        ps_den = psSC.tile([128, 512], F32, tag="pssc",
                                           name="ps_den")
                        nc.tensor.matmul(ps_den[:], ones_m[:], acc[l][:],
                                         start=True, stop=True)
                        rec_bc = recp.tile([128, 512], F32, tag="rec",
                                           name="rec_bc")
                        nc.vector.reciprocal_approx_fast(rec_bc[:], ps_den[:])
                        rec[l] = rec_bc
                    for l in heads:
                        nc.vector.tensor_mul(OT_sb[:, l, q0:q0 + 512],
                                             ps_ot[l][:], rec[l][:])
                    yield

            def stage_c(tbs, pools, b=b, OT_sb=OT_sb):
                """Generator: wo matmuls for the given token blocks.

                pools: PSUM pools to rotate ps_o through (pass the idle
                attention pools for the cross-batch tail so drain latency is
                fully hidden)."""
                npool = 0
                for nt in range(8):
                    wo_sb = wop.tile([128, 4, 512], BF, tag="wo",
                                     name="wo_sb")
                    nc.sync.dma_start(
                        wo_sb[:],
                        woT.rearrange("(o p) n -> p o n",
                                      p=128)[:, :, nt * 512:(nt + 1) * 512])
                    for i, tb in enumerate(tbs):
                        pool, ptag = pools[npool % len(pools)]
                        npool += 1
                        ps_o = pool.tile([128, 512], F32, tag=ptag,
                                         name="ps_o")
                        for k in range(4):
                            nc.tensor.matmul(
                                ps_o[:], OT_sb[:, k, tb * 128:(tb + 1) * 128],
                                wo_sb[:, k, :], start=(k == 0), stop=(k == 3))
                        o_sb = outp.tile([128, 512], F32, tag="o",
                                         name="o_sb")
                        if tb % 2 == 0:
                            nc.scalar.copy(o_sb[:], ps_o[:])
                        else:
                            nc.vector.tensor_copy(o_sb[:], ps_o[:])
                        nc.sync.dma_start(
                            y[b * S + tb * 128:b * S + (tb + 1) * 128,
                              nt * 512:(nt + 1) * 512], o_sb[:])
                        if i % 2 == 1:
                            yield
                    yield

            # ---- batch schedule ----
            # stage A tt=0,1 zipped with the previous batch's stage-C tail
            def a_front():
                for u in stage_a(0, None if b == 0 else (b, 1)):
                    yield u
                for u in stage_a(1, (b, 2)):
                    yield u

            if pending_c_tail is not None:
                _zip_chunks(a_front(), pending_c_tail)
                pending_c_tail = None
            else:
                for _ in a_front():
                    pass
            # attention qh=0 zipped with stage A tt=2,3
            def a_tail():
                for u in stage_a(2, (b, 3)):
                    yield u
                nxt = (b + 1, 0) if b + 1 < B else None
                for u in stage_a(3, nxt):
                    yield u
            _zip_chunks(a_tail(), attn_half(0), lead=3)
            # attention qh=1 zipped with stage C for its ready token blocks
            _zip_chunks(stage_c([0, 1, 2, 3], [(psA, "psa")]), attn_half(1), lead=2)
            # the tb>=4 tail runs zipped into the next batch's stage A,
            # drawing PSUM from the attention pools (idle in that window)
            pending_c_tail = stage_c(
                [4, 5, 6, 7],
                [(psSC, "pssc"), (psOT, "psot")] + ([(psA, "psa")] if b == B - 1 else []))

        if pending_c_tail is not None:
            for _ in pending_c_tail:
                pass

    nc.compile()
    return nc


_CACHE = {}


def _get_program(causal):
    if causal not in _CACHE:
        _CACHE[causal] = _build_program(causal)
    return _CACHE[causal]


def kernel(x, wq_w, wq_a, wq_b, wk_w, wv_w, wv_a, wv_b, wo_w,
           freqs_cos, freqs_sin, mask, start_pos=0, _trace=False):
    assert int(np.asarray(start_pos)) == 0
    shared, cores, causal = _host_prep(
        x, wq_w, wq_a, wq_b, wk_w, wv_w, wv_a, wv_b, wo_w,
        freqs_cos, freqs_sin, mask)
    nc = _get_program(causal)
    in_maps = []
    for c in range(N_CORES):
        m = dict(xT=shared["xT"], cc=shared["cc"], ss=shared["ss"],
                 maskp=shared["maskp"])
        m.update(cores[c])
        in_maps.append(m)
    res = run_bass_kernel_spmd(nc, in_maps, list(range(N_CORES)),
                               trace=_trace)
    kernel._last_results = res
    acc = np.zeros((T, D), np.float32)
    for c in range(N_CORES):
        acc += np.asarray(res.results[c]["y"], np.float32)
    out = acc.reshape(B, S, D)
    return out.astype(np.asarray(x).dtype, copy=False)


# revision 21
# speedup vs baseline: 1.0422x; 1.0422x over previous
"""Trainium2 Bass kernel for LoRA attention prefill (B=4, S=1024, D=4096, H=32).

Sharding: tensor-parallel over heads. Each of the 8 cores computes 4 heads
(512 of the 4096 q/k/v features, column-shard of wq/wk/wv) and a row-shard
of wo, producing a full-shape [T, D] partial output; partials are summed on
the host.

Design notes:
  - LoRA folded into wq/wv on the host (exact algebra) - no device LoRA work.
  - Causal masking is multiplicative (0/1 bf16 after exp); diagonal score
    blocks only compute the live query range (partial-width matmuls).
  - Softmax denominators: exp tiles accumulated on DVE into a bf16 SBUF acc;
    one ones-matrix matmul per head broadcasts the denominator to all 128
    partitions; reciprocal_approx_fast (DVE) replaces the slow serial
    RECIPROCAL.
  - The PE instruction stream is software-pipelined end to end: attention
    rounds (which are exp/Scalar-latency bound) are interleaved with stage-A
    projection and stage-C wo matmul chunks via generators, so the in-order
    PE queue always has independent work. PV matmuls lag two rounds behind
    their score matmuls.
  - Startup weight DMAs are split into consumption-order chunks; the next
    batch's first x tile is prefetched before stage C.
"""
import sys
from contextlib import ExitStack

sys.path.insert(0, "/opt/trn_rl_repo")

import numpy as np
import ml_dtypes

import concourse.bass as bass
import concourse.mybir as mybir
import concourse.tile as tile
from concourse import bacc
from concourse.bass_utils import run_bass_kernel_spmd
from concourse.tile import TileContext

B, S, D = 4, 1024, 4096
H, HD = 32, 128
R = 16
LORA_SCALE = 2.0
N_CORES = 8
HPC = H // N_CORES            # heads per core
FPC = HPC * HD                # features per core = 512
T = B * S                     # 4096 tokens
TT = 256                      # stage-A T-tile (tokens)
NTT = S // TT                 # T-tiles per batch = 4
SCALE = float(1.0 / np.sqrt(HD))
BF = mybir.dt.bfloat16
F32 = mybir.dt.float32


def _bf(a):
    return np.ascontiguousarray(np.asarray(a, np.float32).astype(ml_dtypes.bfloat16))


def _core_perm(c):
    hs = [HPC * c + i for i in range(HPC)]
    ev = np.arange(0, HD, 2)
    od = np.arange(1, HD, 2)
    out = []
    for pair in (0, 1):
        h0, h1 = hs[2 * pair], hs[2 * pair + 1]
        out.append(h0 * HD + ev)
        out.append(h1 * HD + ev)
        out.append(h0 * HD + od)
        out.append(h1 * HD + od)
    return np.concatenate(out)


def _check_causal(mask):
    iu = np.triu_indices(S, k=1)
    il = np.tril_indices(S, k=0)
    return bool((mask[iu] <= -1e8).all() and (mask[il] == 0).all())


def _host_prep(x, wq_w, wq_a, wq_b, wk_w, wv_w, wv_a, wv_b, wo_w,
               freqs_cos, freqs_sin, mask):
    x2 = np.asarray(x, np.float32).reshape(T, D)
    xT = _bf(x2.T)

    # fold LoRA into the dense weights: y = x(W + s*B@A)^T exactly
    wq_eff = np.asarray(wq_w, np.float32) + LORA_SCALE * (
        np.asarray(wq_b, np.float32) @ np.asarray(wq_a, np.float32))
    wv_eff = np.asarray(wv_w, np.float32) + LORA_SCALE * (
        np.asarray(wv_b, np.float32) @ np.asarray(wv_a, np.float32))
    wk = np.asarray(wk_w, np.float32)

    cosT = np.asarray(freqs_cos, np.float32).T
    sinT = np.asarray(freqs_sin, np.float32).T
    cc = np.ascontiguousarray(np.tile(cosT, (2, B)).astype(np.float32))
    ss = np.ascontiguousarray(np.tile(sinT, (2, B)).astype(np.float32))

    mask = np.asarray(mask, np.float32)
    causal = _check_causal(mask)
    if causal:
        # 0/1 multiplicative triangle for the 128x128 diagonal strips
        tri = np.tril(np.ones((128, 128), np.float32)).T  # [k,q]: 1 if k<=q
        maskp = _bf(np.broadcast_to(tri, (4, 128, 128)))
    else:
        mT = mask.T * np.float32(np.sqrt(HD))
        maskp = np.zeros((8, 128, 2, 512), np.float32)
        for qh in range(2):
            for j in range(8):
                maskp[j, :, qh, :] = mT[j * 128:(j + 1) * 128,
                                        qh * 512:(qh + 1) * 512]

    shared = dict(xT=xT, cc=cc, ss=ss, maskp=maskp)
    cores = []
    for c in range(N_CORES):
        perm = _core_perm(c)
        sl = slice(c * FPC, (c + 1) * FPC)
        cores.append(dict(
            wqT=_bf(wq_eff[perm, :].T),
            wkT=_bf(wk[perm, :].T),
            wvT=_bf(wv_eff[sl, :].T),
            woT=_bf(np.asarray(wo_w, np.float32)[:, sl].T),
        ))
    return shared, cores, causal


def _zip_chunks(*gens, lead=0):
    """Round-robin drive generators to completion. `lead` pulls that many
    chunks from the first generator before starting the round-robin."""
    gens = list(gens)
    for _ in range(lead):
        try:
            next(gens[0])
        except StopIteration:
            gens.pop(0)
            break
    while gens:
        for g in list(gens):
            try:
                next(g)
            except StopIteration:
                gens.remove(g)


def _build_program(causal):
    nc = bacc.Bacc("TRN2", num_devices=N_CORES)

    xT = nc.dram_tensor("xT", [D, T], BF, kind="ExternalInput").ap()
    wqT = nc.dram_tensor("wqT", [D, FPC], BF, kind="ExternalInput").ap()
    wkT = nc.dram_tensor("wkT", [D, FPC], BF, kind="ExternalInput").ap()
    wvT = nc.dram_tensor("wvT", [D, FPC], BF, kind="ExternalInput").ap()
    woT = nc.dram_tensor("woT", [FPC, D], BF, kind="ExternalInput").ap()
    cc = nc.dram_tensor("cc", [128, T], F32, kind="ExternalInput").ap()
    ss = nc.dram_tensor("ss", [128, T], F32, kind="ExternalInput").ap()
    if causal:
        maskp = nc.dram_tensor("maskp", [4, 128, 128], BF,
                               kind="ExternalInput").ap()
    else:
        maskp = nc.dram_tensor("maskp", [8, 128, 2, 512], F32,
                               kind="ExternalInput").ap()
    y = nc.dram_tensor("y", [T, D], F32, kind="ExternalOutput").ap()

    with TileContext(nc) as tc, ExitStack() as ctx:
        wpool = ctx.enter_context(tc.tile_pool(name="wpool", bufs=1))
        xpool = ctx.enter_context(tc.tile_pool(name="xpool", bufs=2))
        ccp = ctx.enter_context(tc.tile_pool(name="ccp", bufs=4))
        qkvp = ctx.enter_context(tc.tile_pool(name="qkvp", bufs=1))
        expp = ctx.enter_context(tc.tile_pool(name="expp", bufs=7))
        accp = ctx.enter_context(tc.tile_pool(name="accp", bufs=4))
        recp = ctx.enter_context(tc.tile_pool(name="recp", bufs=2))
        otp = ctx.enter_context(tc.tile_pool(name="otp", bufs=1))
        outp = ctx.enter_context(tc.tile_pool(name="outp", bufs=4))
        tmpp = ctx.enter_context(tc.tile_pool(name="tmpp", bufs=4))
        stp = ctx.enter_context(tc.tile_pool(name="stp", bufs=4))
        wop = ctx.enter_context(tc.tile_pool(name="wop", bufs=2))
        psA = ctx.enter_context(tc.tile_pool(name="psA", bufs=3, space="PSUM"))
        psOT = ctx.enter_context(tc.tile_pool(name="psOT", bufs=2,
                                              space="PSUM"))
        psSC = ctx.enter_context(tc.tile_pool(name="psSC", bufs=3,
                                              space="PSUM"))

        xre = xT.rearrange("(o p) t -> p o t", p=128)
        # prefetched stage-A input tiles, keyed by (b, tt)
        fetched = {}

        def fetch_x(b, tt, xq=None):
            t0 = b * S + tt * TT
            x_sb = xpool.tile([128, 32, TT], BF, tag="x", name="x_sb")
            (xq or nc.sync).dma_start(x_sb[:], xre[:, :, t0:t0 + TT])
            cc_sb = ccp.tile([128, TT], F32, tag="cc", name="cc_sb")
            nc.sync.dma_start(cc_sb[:], cc[:, t0:t0 + TT])
            ss_sb = ccp.tile([128, TT], F32, tag="ss", name="ss_sb")
            nc.sync.dma_start(ss_sb[:], ss[:, t0:t0 + TT])
            fetched[(b, tt)] = (x_sb, cc_sb, ss_sb)

        # first input tile before the weights so compute starts early
        fetch_x(0, 0)

        # resident weights, split into consumption-order chunks so the first
        # projection matmuls can start before the full weight set has landed
        wq_sb = wpool.tile([128, 32, FPC], BF, tag="wq")
        wk_sb = wpool.tile([128, 32, FPC], BF, tag="wk")
        wv_sb = wpool.tile([128, 32, FPC], BF, tag="wv")
        wqr = wqT.rearrange("(o p) f -> p o f", p=128)
        wkr = wkT.rearrange("(o p) f -> p o f", p=128)
        wvr = wvT.rearrange("(o p) f -> p o f", p=128)
        # spread the one-time weight loads over all three DMA lanes in
        # first-use order: sync also carries the x tiles, scalar must be
        # clear before tt0's RoPE regroup copies, gpsimd is the slow bulk
        # lane for whatever has the latest deadline
        nc.scalar.dma_start(wq_sb[:, :, 128:256], wqr[:, :, 128:256])
        nc.scalar.dma_start(wk_sb[:, :, 128:256], wkr[:, :, 128:256])
        nc.sync.dma_start(wq_sb[:, :, 0:128], wqr[:, :, 0:128])
        nc.sync.dma_start(wk_sb[:, :, 0:128], wkr[:, :, 0:128])
        nc.gpsimd.dma_start(wq_sb[:, :, 256:384], wqr[:, :, 256:384])
        nc.gpsimd.dma_start(wq_sb[:, :, 384:512], wqr[:, :, 384:512])
        nc.gpsimd.dma_start(wk_sb[:, :, 256:384], wkr[:, :, 256:384])
        nc.gpsimd.dma_start(wk_sb[:, :, 384:512], wkr[:, :, 384:512])
        nc.sync.dma_start(wv_sb[:, :, 0:256], wvr[:, :, 0:256])
        nc.gpsimd.dma_start(wv_sb[:, :, 256:512], wvr[:, :, 256:512])
        if causal:
            mask_sb = wpool.tile([128, 4, 128], BF, tag="mask")
            nc.gpsimd.dma_start(mask_sb[:], maskp.rearrange("j p n -> p j n"))
        else:
            mask_sb = wpool.tile([128, 8, 2, 512], F32, tag="mask")
            nc.sync.dma_start(mask_sb[:],
                              maskp.rearrange("j p q n -> p j q n"))
        ones_m = wpool.tile([128, 128], BF, tag="onesm")
        nc.gpsimd.memset(ones_m[:], 1.0)
        fetch_x(0, 1)

        pending_c_tail = None
        for b in range(B):
            Q_sb = qkvp.tile([128, 4, S], BF, tag="Qsb")
            K_sb = qkvp.tile([128, 4, S], BF, tag="Ksb")
            V_sb = qkvp.tile([128, 8, FPC], BF, tag="Vsb")
            OT_sb = otp.tile([128, 4, S], BF, tag="OT")

            def stage_a(tt, prefetch_next):
                """Generator: QK pair groups + V blocks for one t-tile."""
                x_sb, cc_sb, ss_sb = fetched.pop((b, tt))
                if prefetch_next is not None:
                    fetch_x(*prefetch_next)
                toff = tt * TT
                for dst_sb, w_sb in ((Q_sb, wq_sb), (K_sb, wk_sb)):
                    for pair in range(2):
                        # one PSUM bank holds both the real and imag halves
                        # (the r accumulation group completes before the i
                        # group's start clears the bank's has_written bits)
                        ps = psA.tile([128, 512], F32, tag="psa",
                                      name="ps_qk")
                        for ri in range(2):
                            f0 = pair * 256 + ri * 128
                            for d in range(32):
                                nc.tensor.matmul(
                                    ps[:, ri * TT:ri * TT + TT],
                                    w_sb[:, d, f0:f0 + 128],
                                    x_sb[:, d, :], start=(d == 0),
                                    stop=(d == 31))
                        ps_r = ps[:, 0:TT]
                        ps_i = ps[:, TT:2 * TT]
                        t1 = tmpp.tile([128, TT], F32, tag="t", name="t1")
                        nc.vector.tensor_mul(t1[:], ps_r, cc_sb[:])
                        t2 = tmpp.tile([128, TT], F32, tag="t", name="t2")
                        nc.vector.tensor_mul(t2[:], ps_i, ss_sb[:])
                        st_r = stp.tile([128, TT], BF, tag="st", name="st_r")
                        nc.vector.tensor_tensor(
                            st_r[:], t1[:], t2[:], mybir.AluOpType.subtract)
                        t3 = tmpp.tile([128, TT], F32, tag="t", name="t3")
                        nc.vector.tensor_mul(t3[:], ps_r, ss_sb[:])
                        t4 = tmpp.tile([128, TT], F32, tag="t", name="t4")
                        nc.vector.tensor_mul(t4[:], ps_i, cc_sb[:])
                        st_i = stp.tile([128, TT], BF, tag="st", name="st_i")
                        nc.vector.tensor_tensor(
                            st_i[:], t3[:], t4[:], mybir.AluOpType.add)
                        # shuffle into head-contiguous blocks: head h of this
                        # pair = [r half; i half] on partitions [0:64|64:128]
                        for hh in range(2):
                            h_loc = 2 * pair + hh
                            nc.scalar.dma_start(
                                dst_sb[0:64, h_loc, toff:toff + TT],
                                st_r[hh * 64:(hh + 1) * 64, :])
                            nc.scalar.dma_start(
                                dst_sb[64:128, h_loc, toff:toff + TT],
                                st_i[hh * 64:(hh + 1) * 64, :])
                        yield
                for v in range(TT // 128):
                    tb = tt * (TT // 128) + v
                    ps_v = psA.tile([128, 512], F32, tag="psa", name="ps_v")
                    for d in range(32):
                        nc.tensor.matmul(
                            ps_v[:], x_sb[:, d, v * 128:(v + 1) * 128],
                            wv_sb[:, d, :], start=(d == 0), stop=(d == 31))
                    nc.scalar.copy(V_sb[:, tb, :], ps_v[:])
                    yield

            def attn_half(qh):
                """Generator: attention rounds for one query half."""
                q0 = qh * 512
                kbs = list(range(0, qh * 4 + 4)) if causal else list(range(8))
                nkb = len(kbs)
                LAG = 2

                def width(kb):
                    if causal and kb - 4 * qh >= 0:
                        return (kb - 4 * qh) * 128
                    return 0

                for hp in range(2):
                    heads = (2 * hp, 2 * hp + 1)
                    ps_ot = {}
                    acc = {}
                    e_t = {}
                    for l in heads:
                        ps_ot[l] = psOT.tile([128, 512], F32, tag="psot",
                                             name="ps_ot")
                        acc[l] = accp.tile([128, 512], BF, tag="acc",
                                           name="acc")

                    def emit_sc(l, j):
                        kb = kbs[j]
                        qlo = width(kb)
                        ps_sc = psSC.tile([128, 512], F32, tag="pssc",
                                          name="ps_sc")
                        nc.tensor.matmul(
                            ps_sc[:, qlo:512],
                            K_sb[:, l, kb * 128:kb * 128 + 128],
                            Q_sb[:, l, q0 + qlo:q0 + 512],
                            start=True, stop=True)
                        e_sb = expp.tile([128, 512], BF, tag="e", name="e_sb")
                        if causal:
                            nc.scalar.activation(
                                e_sb[:, qlo:512], ps_sc[:, qlo:512],
                                mybir.ActivationFunctionType.Exp, scale=SCALE)
                            jj = kb - 4 * qh
                            if jj >= 0:
                                nc.vector.tensor_mul(
                                    e_sb[:, qlo:qlo + 128],
                                    e_sb[:, qlo:qlo + 128],
                                    mask_sb[:, jj, :])
                        else:
                            nc.vector.tensor_add(
                                ps_sc[:], ps_sc[:], mask_sb[:, kb, qh, :])
                            nc.scalar.activation(
                                e_sb[:], ps_sc[:],
                                mybir.ActivationFunctionType.Exp, scale=SCALE)
                        e_t[(l, j)] = e_sb

                    def emit_pv(l, j):
                        kb = kbs[j]
                        qlo = width(kb)
                        e_sb = e_t.pop((l, j))
                        nc.tensor.matmul(
                            ps_ot[l][:, qlo:512],
                            V_sb[:, kb, l * 128:(l + 1) * 128],
                            e_sb[:, qlo:512],
                            start=(j == 0), stop=(j == nkb - 1))
                        if j == 0:
                            nc.vector.tensor_copy(acc[l][:], e_sb[:])
                        else:
                            nc.vector.tensor_add(
                                acc[l][:, qlo:512], acc[l][:, qlo:512],
                                e_sb[:, qlo:512])

                    # software pipeline: PV lags LAG key blocks behind scores
                    for j in range(nkb):
                        for l in heads:
                            emit_sc(l, j)
                            if j >= LAG:
                                emit_pv(l, j - LAG)
                        yield
                    for j in range(max(nkb - LAG, 0), nkb):
                        for l in heads:
                            emit_pv(l, j)
                    # normalization tail for this head pair
                    rec = {}
                    for l in heads:
                        ps_den = psSC.tile([128, 512], F32, tag="pssc",
                                           name="ps_den")
                        nc.tensor.matmul(ps_den[:], ones_m[:], acc[l][:],
                                         start=True, stop=True)
                        rec_bc = recp.tile([128, 512], F32, tag="rec",
                                           name="rec_bc")
                        nc.vector.reciprocal_approx_fast(rec_bc[:], ps_den[:])
                        rec[l] = rec_bc
                    for l in heads:
                        nc.vector.tensor_mul(OT_sb[:, l, q0:q0 + 512],
                                             ps_ot[l][:], rec[l][:])
                    yield

            def stage_c(tbs, pools, b=b, OT_sb=OT_sb):
                """Generator: wo matmuls for the given token blocks.

                pools: PSUM pools to rotate ps_o through (pass the idle
                attention pools for the cross-batch tail so drain latency is
                fully hidden)."""
                npool = 0
                for nt in range(8):
                    wo_sb = wop.tile([128, 4, 512], BF, tag="wo",
                                     name="wo_sb")
                    nc.sync.dma_start(
                        wo_sb[:],
                        woT.rearrange("(o p) n -> p o n",
                                      p=128)[:, :, nt * 512:(nt + 1) * 512])
                    for i, tb in enumerate(tbs):
                        pool, ptag = pools[npool % len(pools)]
                        npool += 1
                        ps_o = pool.tile([128, 512], F32, tag=ptag,
                                         name="ps_o")
                        for k in range(4):
                            nc.tensor.matmul(
                                ps_o[:], OT_sb[:, k, tb * 128:(tb + 1) * 128],
                                wo_sb[:, k, :], start=(k == 0), stop=(k == 3))
                        o_sb = outp.tile([128, 512], F32, tag="o",
                                         name="o_sb")
                        if tb % 2 == 0:
                            nc.scalar.copy(o_sb[:], ps_o[:])
                        else:
                            nc.vector.tensor_copy(o_sb[:], ps_o[:])
                        nc.sync.dma_start(
                            y[b * S + tb * 128:b * S + (tb + 1) * 128,
                              nt * 512:(nt + 1) * 512], o_sb[:])
                        if i % 2 == 1:
                            yield
                    yield

            # ---- batch schedule ----
            # stage A tt=0,1 zipped with the previous batch's stage-C tail
            def a_front():
                for u in stage_a(0, None if b == 0 else (b, 1)):
                    yield u
                for u in stage_a(1, (b, 2)):
                    yield u

            if pending_c_tail is not None:
                _zip_chunks(a_front(), pending_c_tail)
                pending_c_tail = None
            else:
                for _ in a_front():
                    pass
            # attention qh=0 zipped with stage A tt=2,3
            def a_tail():
                for u in stage_a(2, (b, 3)):
                    yield u
                nxt = (b + 1, 0) if b + 1 < B else None
                for u in stage_a(3, nxt):
                    yield u
            _zip_chunks(a_tail(), attn_half(0), lead=3)
            # attention qh=1 zipped with stage C for its ready token blocks
            _zip_chunks(stage_c([0, 1, 2, 3], [(psA, "psa")]), attn_half(1), lead=2)
            # the tb>=4 tail runs zipped into the next batch's stage A,
            # drawing PSUM from the attention pools (idle in that window)
            pending_c_tail = stage_c(
                [4, 5, 6, 7],
                [(psSC, "pssc"), (psOT, "psot")] + ([(psA, "psa")] if b == B - 1 else []))

        if pending_c_tail is not None:
            for _ in pending_c_tail:
                pass

    nc.compile()
    return nc


_CACHE = {}


def _get_program(causal):
    if causal not in _CACHE:
        _CACHE[causal] = _build_program(causal)
    return _CACHE[causal]


def kernel(x, wq_w, wq_a, wq_b, wk_w, wv_w, wv_a, wv_b, wo_w,
           freqs_cos, freqs_sin, mask, start_pos=0, _trace=False):
    assert int(np.asarray(start_pos)) == 0
    shared, cores, causal = _host_prep(
        x, wq_w, wq_a, wq_b, wk_w, wv_w, wv_a, wv_b, wo_w,
        freqs_cos, freqs_sin, mask)
    nc = _get_program(causal)
    in_maps = []
    for c in range(N_CORES):
        m = dict(xT=shared["xT"], cc=shared["cc"], ss=shared["ss"],
                 maskp=shared["maskp"])
        m.update(cores[c])
        in_maps.append(m)
    res = run_bass_kernel_spmd(nc, in_maps, list(range(N_CORES)),
                               trace=_trace)
    kernel._last_results = res
    acc = np.zeros((T, D), np.float32)
    for c in range(N_CORES):
        acc += np.asarray(res.results[c]["y"], np.float32)
    out = acc.reshape(B, S, D)
    return out.astype(np.asarray(x).dtype, copy=False)


# revision 22
# speedup vs baseline: 1.0844x; 1.0406x over previous
"""Trainium2 Bass kernel for LoRA attention prefill (B=4, S=1024, D=4096, H=32).

Sharding: tensor-parallel over heads. Each of the 8 cores computes 4 heads
(512 of the 4096 q/k/v features, column-shard of wq/wk/wv) and a row-shard
of wo, producing a full-shape [T, D] partial output; partials are summed on
the host.

Design notes:
  - LoRA folded into wq/wv on the host (exact algebra) - no device LoRA work.
  - Causal masking is multiplicative (0/1 bf16 after exp); diagonal score
    blocks only compute the live query range (partial-width matmuls).
  - Softmax denominators: exp tiles accumulated on DVE into a bf16 SBUF acc;
    one ones-matrix matmul per head broadcasts the denominator to all 128
    partitions; reciprocal_approx_fast (DVE) replaces the slow serial
    RECIPROCAL.
  - The PE instruction stream is software-pipelined end to end: attention
    rounds (which are exp/Scalar-latency bound) are interleaved with stage-A
    projection and stage-C wo matmul chunks via generators, so the in-order
    PE queue always has independent work. PV matmuls lag two rounds behind
    their score matmuls.
  - Startup weight DMAs are split into consumption-order chunks; the next
    batch's first x tile is prefetched before stage C.
"""
import sys
from contextlib import ExitStack

sys.path.insert(0, "/opt/trn_rl_repo")

import numpy as np
import ml_dtypes

import concourse.bass as bass
import concourse.mybir as mybir
import concourse.tile as tile
from concourse import bacc
from concourse.bass_utils import run_bass_kernel_spmd
from concourse.tile import TileContext

B, S, D = 4, 1024, 4096
H, HD = 32, 128
R = 16
LORA_SCALE = 2.0
N_CORES = 8
HPC = H // N_CORES            # heads per core
FPC = HPC * HD                # features per core = 512
T = B * S                     # 4096 tokens
TT = 256                      # stage-A T-tile (tokens)
NTT = S // TT                 # T-tiles per batch = 4
SCALE = float(1.0 / np.sqrt(HD))
BF = mybir.dt.bfloat16
F32 = mybir.dt.float32


def _bf(a):
    return np.ascontiguousarray(np.asarray(a, np.float32).astype(ml_dtypes.bfloat16))


def _core_perm(c):
    hs = [HPC * c + i for i in range(HPC)]
    ev = np.arange(0, HD, 2)
    od = np.arange(1, HD, 2)
    out = []
    for pair in (0, 1):
        h0, h1 = hs[2 * pair], hs[2 * pair + 1]
        out.append(h0 * HD + ev)
        out.append(h1 * HD + ev)
        out.append(h0 * HD + od)
        out.append(h1 * HD + od)
    return np.concatenate(out)


def _check_causal(mask):
    iu = np.triu_indices(S, k=1)
    il = np.tril_indices(S, k=0)
    return bool((mask[iu] <= -1e8).all() and (mask[il] == 0).all())


def _host_prep(x, wq_w, wq_a, wq_b, wk_w, wv_w, wv_a, wv_b, wo_w,
               freqs_cos, freqs_sin, mask):
    x2 = np.asarray(x, np.float32).reshape(T, D)
    # [p, tile, o, t]: per-partition-contiguous x tiles for full-speed DMA
    xR = _bf(np.ascontiguousarray(
        x2.T.reshape(32, 128, T // TT, TT).transpose(1, 2, 0, 3)))

    # fold LoRA into the dense weights: y = x(W + s*B@A)^T exactly
    wq_eff = np.asarray(wq_w, np.float32) + LORA_SCALE * (
        np.asarray(wq_b, np.float32) @ np.asarray(wq_a, np.float32))
    wv_eff = np.asarray(wv_w, np.float32) + LORA_SCALE * (
        np.asarray(wv_b, np.float32) @ np.asarray(wv_a, np.float32))
    wk = np.asarray(wk_w, np.float32)

    cosT = np.asarray(freqs_cos, np.float32).T
    sinT = np.asarray(freqs_sin, np.float32).T
    cc = np.ascontiguousarray(np.tile(cosT, (2, B)).astype(np.float32))
    ss = np.ascontiguousarray(np.tile(sinT, (2, B)).astype(np.float32))

    mask = np.asarray(mask, np.float32)
    causal = _check_causal(mask)
    if causal:
        # 0/1 multiplicative triangle for the 128x128 diagonal strips
        tri = np.tril(np.ones((128, 128), np.float32)).T  # [k,q]: 1 if k<=q
        maskp = _bf(np.broadcast_to(tri, (4, 128, 128)))
    else:
        mT = mask.T * np.float32(np.sqrt(HD))
        maskp = np.zeros((8, 128, 2, 512), np.float32)
        for qh in range(2):
            for j in range(8):
                maskp[j, :, qh, :] = mT[j * 128:(j + 1) * 128,
                                        qh * 512:(qh + 1) * 512]

    shared = dict(xR=xR, cc=cc, ss=ss, maskp=maskp)
    cores = []
    for c in range(N_CORES):
        perm = _core_perm(c)
        sl = slice(c * FPC, (c + 1) * FPC)
        wqT = wq_eff[perm, :].T
        wkT = wk[perm, :].T
        wvT = wv_eff[sl, :].T
        woT = np.asarray(wo_w, np.float32)[:, sl].T
        cores.append(dict(
            wqR=_bf(np.ascontiguousarray(
                wqT.reshape(32, 128, 4, 128).transpose(1, 2, 0, 3))),
            wkR=_bf(np.ascontiguousarray(
                wkT.reshape(32, 128, 4, 128).transpose(1, 2, 0, 3))),
            wvR=_bf(np.ascontiguousarray(
                wvT.reshape(32, 128, 512).transpose(1, 0, 2))),
            woR=_bf(np.ascontiguousarray(
                woT.reshape(4, 128, 8, 512).transpose(1, 2, 0, 3))),
        ))
    return shared, cores, causal


def _zip_chunks(*gens, lead=0):
    """Round-robin drive generators to completion. `lead` pulls that many
    chunks from the first generator before starting the round-robin."""
    gens = list(gens)
    for _ in range(lead):
        try:
            next(gens[0])
        except StopIteration:
            gens.pop(0)
            break
    while gens:
        for g in list(gens):
            try:
                next(g)
            except StopIteration:
                gens.remove(g)


def _build_program(causal):
    nc = bacc.Bacc("TRN2", num_devices=N_CORES)

    xR = nc.dram_tensor("xR", [128, T // TT, 32, TT], BF,
                        kind="ExternalInput").ap()
    wqR = nc.dram_tensor("wqR", [128, 4, 32, 128], BF,
                         kind="ExternalInput").ap()
    wkR = nc.dram_tensor("wkR", [128, 4, 32, 128], BF,
                         kind="ExternalInput").ap()
    wvR = nc.dram_tensor("wvR", [128, 32, 512], BF,
                         kind="ExternalInput").ap()
    woR = nc.dram_tensor("woR", [128, 8, 4, 512], BF,
                         kind="ExternalInput").ap()
    cc = nc.dram_tensor("cc", [128, T], F32, kind="ExternalInput").ap()
    ss = nc.dram_tensor("ss", [128, T], F32, kind="ExternalInput").ap()
    if causal:
        maskp = nc.dram_tensor("maskp", [4, 128, 128], BF,
                               kind="ExternalInput").ap()
    else:
        maskp = nc.dram_tensor("maskp", [8, 128, 2, 512], F32,
                               kind="ExternalInput").ap()
    y = nc.dram_tensor("y", [T, D], F32, kind="ExternalOutput").ap()

    with TileContext(nc) as tc, ExitStack() as ctx:
        wpool = ctx.enter_context(tc.tile_pool(name="wpool", bufs=1))
        xpool = ctx.enter_context(tc.tile_pool(name="xpool", bufs=2))
        ccp = ctx.enter_context(tc.tile_pool(name="ccp", bufs=4))
        qkvp = ctx.enter_context(tc.tile_pool(name="qkvp", bufs=1))
        expp = ctx.enter_context(tc.tile_pool(name="expp", bufs=7))
        accp = ctx.enter_context(tc.tile_pool(name="accp", bufs=4))
        recp = ctx.enter_context(tc.tile_pool(name="recp", bufs=2))
        otp = ctx.enter_context(tc.tile_pool(name="otp", bufs=1))
        outp = ctx.enter_context(tc.tile_pool(name="outp", bufs=4))
        tmpp = ctx.enter_context(tc.tile_pool(name="tmpp", bufs=4))
        stp = ctx.enter_context(tc.tile_pool(name="stp", bufs=4))
        wop = ctx.enter_context(tc.tile_pool(name="wop", bufs=2))
        psA = ctx.enter_context(tc.tile_pool(name="psA", bufs=3, space="PSUM"))
        psOT = ctx.enter_context(tc.tile_pool(name="psOT", bufs=2,
                                              space="PSUM"))
        psSC = ctx.enter_context(tc.tile_pool(name="psSC", bufs=3,
                                              space="PSUM"))

        # prefetched stage-A input tiles, keyed by (b, tt)
        fetched = {}

        def fetch_x(b, tt, xq=None):
            t0 = b * S + tt * TT
            x_sb = xpool.tile([128, 32, TT], BF, tag="x", name="x_sb")
            (xq or nc.sync).dma_start(x_sb[:], xR[:, t0 // TT])
            cc_sb = ccp.tile([128, TT], F32, tag="cc", name="cc_sb")
            nc.sync.dma_start(cc_sb[:], cc[:, t0:t0 + TT])
            ss_sb = ccp.tile([128, TT], F32, tag="ss", name="ss_sb")
            nc.sync.dma_start(ss_sb[:], ss[:, t0:t0 + TT])
            fetched[(b, tt)] = (x_sb, cc_sb, ss_sb)

        # first input tile before the weights so compute starts early
        fetch_x(0, 0)

        # resident weights, chunk-major so every DMA is a contiguous
        # per-partition copy; spread over the three DMA lanes in first-use
        # order (scalar must be clear before tt0's RoPE regroup copies)
        wq_sb = wpool.tile([128, 4, 32, 128], BF, tag="wq")
        wk_sb = wpool.tile([128, 4, 32, 128], BF, tag="wk")
        wv_sb = wpool.tile([128, 32, FPC], BF, tag="wv")
        nc.scalar.dma_start(wq_sb[:, 1], wqR[:, 1])
        nc.scalar.dma_start(wk_sb[:, 1], wkR[:, 1])
        nc.sync.dma_start(wq_sb[:, 0], wqR[:, 0])
        nc.sync.dma_start(wk_sb[:, 0], wkR[:, 0])
        nc.gpsimd.dma_start(wq_sb[:, 2], wqR[:, 2])
        nc.gpsimd.dma_start(wq_sb[:, 3], wqR[:, 3])
        nc.gpsimd.dma_start(wk_sb[:, 2], wkR[:, 2])
        nc.gpsimd.dma_start(wk_sb[:, 3], wkR[:, 3])
        nc.sync.dma_start(wv_sb[:, 0:16, :], wvR[:, 0:16, :])
        nc.gpsimd.dma_start(wv_sb[:, 16:32, :], wvR[:, 16:32, :])
        if causal:
            mask_sb = wpool.tile([128, 4, 128], BF, tag="mask")
            nc.gpsimd.dma_start(mask_sb[:], maskp.rearrange("j p n -> p j n"))
        else:
            mask_sb = wpool.tile([128, 8, 2, 512], F32, tag="mask")
            nc.sync.dma_start(mask_sb[:],
                              maskp.rearrange("j p q n -> p j q n"))
        ones_m = wpool.tile([128, 128], BF, tag="onesm")
        nc.gpsimd.memset(ones_m[:], 1.0)
        fetch_x(0, 1)

        pending_c_tail = None
        for b in range(B):
            Q_sb = qkvp.tile([128, 4, S], BF, tag="Qsb")
            K_sb = qkvp.tile([128, 4, S], BF, tag="Ksb")
            V_sb = qkvp.tile([128, 8, FPC], BF, tag="Vsb")
            OT_sb = otp.tile([128, 4, S], BF, tag="OT")

            def stage_a(tt, prefetch_next):
                """Generator: QK pair groups + V blocks for one t-tile."""
                x_sb, cc_sb, ss_sb = fetched.pop((b, tt))
                if prefetch_next is not None:
                    fetch_x(*prefetch_next)
                toff = tt * TT
                for dst_sb, w_sb in ((Q_sb, wq_sb), (K_sb, wk_sb)):
                    for pair in range(2):
                        # one PSUM bank holds both the real and imag halves
                        # (the r accumulation group completes before the i
                        # group's start clears the bank's has_written bits)
                        ps = psA.tile([128, 512], F32, tag="psa",
                                      name="ps_qk")
                        for ri in range(2):
                            c = pair * 2 + ri
                            for d in range(32):
                                nc.tensor.matmul(
                                    ps[:, ri * TT:ri * TT + TT],
                                    w_sb[:, c, d, :],
                                    x_sb[:, d, :], start=(d == 0),
                                    stop=(d == 31))
                        ps_r = ps[:, 0:TT]
                        ps_i = ps[:, TT:2 * TT]
                        t1 = tmpp.tile([128, TT], F32, tag="t", name="t1")
                        nc.vector.tensor_mul(t1[:], ps_r, cc_sb[:])
                        t2 = tmpp.tile([128, TT], F32, tag="t", name="t2")
                        nc.vector.tensor_mul(t2[:], ps_i, ss_sb[:])
                        st_r = stp.tile([128, TT], BF, tag="st", name="st_r")
                        nc.vector.tensor_tensor(
                            st_r[:], t1[:], t2[:], mybir.AluOpType.subtract)
                        t3 = tmpp.tile([128, TT], F32, tag="t", name="t3")
                        nc.vector.tensor_mul(t3[:], ps_r, ss_sb[:])
                        t4 = tmpp.tile([128, TT], F32, tag="t", name="t4")
                        nc.vector.tensor_mul(t4[:], ps_i, cc_sb[:])
                        st_i = stp.tile([128, TT], BF, tag="st", name="st_i")
                        nc.vector.tensor_tensor(
                            st_i[:], t3[:], t4[:], mybir.AluOpType.add)
                        # shuffle into head-contiguous blocks: head h of this
                        # pair = [r half; i half] on partitions [0:64|64:128]
                        for hh in range(2):
                            h_loc = 2 * pair + hh
                            nc.scalar.dma_start(
                                dst_sb[0:64, h_loc, toff:toff + TT],
                                st_r[hh * 64:(hh + 1) * 64, :])
                            nc.scalar.dma_start(
                                dst_sb[64:128, h_loc, toff:toff + TT],
                                st_i[hh * 64:(hh + 1) * 64, :])
                        yield
                for v in range(TT // 128):
                    tb = tt * (TT // 128) + v
                    ps_v = psA.tile([128, 512], F32, tag="psa", name="ps_v")
                    for d in range(32):
                        nc.tensor.matmul(
                            ps_v[:], x_sb[:, d, v * 128:(v + 1) * 128],
                            wv_sb[:, d, :], start=(d == 0), stop=(d == 31))
                    nc.scalar.copy(V_sb[:, tb, :], ps_v[:])
                    yield

            def attn_half(qh):
                """Generator: attention rounds for one query half."""
                q0 = qh * 512
                kbs = list(range(0, qh * 4 + 4)) if causal else list(range(8))
                nkb = len(kbs)
                LAG = 2

                def width(kb):
                    if causal and kb - 4 * qh >= 0:
                        return (kb - 4 * qh) * 128
                    return 0

                for hp in range(2):
                    heads = (2 * hp, 2 * hp + 1)
                    ps_ot = {}
                    acc = {}
                    e_t = {}
                    for l in heads:
                        ps_ot[l] = psOT.tile([128, 512], F32, tag="psot",
                                             name="ps_ot")
                        acc[l] = accp.tile([128, 512], BF, tag="acc",
                                           name="acc")

                    def emit_sc(l, j):
                        kb = kbs[j]
                        qlo = width(kb)
                        ps_sc = psSC.tile([128, 512], F32, tag="pssc",
                                          name="ps_sc")
                        nc.tensor.matmul(
                            ps_sc[:, qlo:512],
                            K_sb[:, l, kb * 128:kb * 128 + 128],
                            Q_sb[:, l, q0 + qlo:q0 + 512],
                            start=True, stop=True)
                        e_sb = expp.tile([128, 512], BF, tag="e", name="e_sb")
                        if causal:
                            nc.scalar.activation(
                                e_sb[:, qlo:512], ps_sc[:, qlo:512],
                                mybir.ActivationFunctionType.Exp, scale=SCALE)
                            jj = kb - 4 * qh
                            if jj >= 0:
                                nc.vector.tensor_mul(
                                    e_sb[:, qlo:qlo + 128],
                                    e_sb[:, qlo:qlo + 128],
                                    mask_sb[:, jj, :])
                        else:
                            nc.vector.tensor_add(
                                ps_sc[:], ps_sc[:], mask_sb[:, kb, qh, :])
                            nc.scalar.activation(
                                e_sb[:], ps_sc[:],
                                mybir.ActivationFunctionType.Exp, scale=SCALE)
                        e_t[(l, j)] = e_sb

                    def emit_pv(l, j):
                        kb = kbs[j]
                        qlo = width(kb)
                        e_sb = e_t.pop((l, j))
                        nc.tensor.matmul(
                            ps_ot[l][:, qlo:512],
                            V_sb[:, kb, l * 128:(l + 1) * 128],
                            e_sb[:, qlo:512],
                            start=(j == 0), stop=(j == nkb - 1))
                        if j == 0:
                            nc.vector.tensor_copy(acc[l][:], e_sb[:])
                        else:
                            nc.vector.tensor_add(
                                acc[l][:, qlo:512], acc[l][:, qlo:512],
                                e_sb[:, qlo:512])

                    # software pipeline: PV lags LAG key blocks behind scores
                    for j in range(nkb):
                        for l in heads:
                            emit_sc(l, j)
                            if j >= LAG:
                                emit_pv(l, j - LAG)
                        yield
                    for j in range(max(nkb - LAG, 0), nkb):
                        for l in heads:
                            emit_pv(l, j)
                    # normalization tail for this head pair
                    rec = {}
                    for l in heads:
                        ps_den = psSC.tile([128, 512], F32, tag="pssc",
                                           name="ps_den")
                        nc.tensor.matmul(ps_den[:], ones_m[:], acc[l][:],
                                         start=True, stop=True)
                        rec_bc = recp.tile([128, 512], F32, tag="rec",
                                           name="rec_bc")
                        nc.vector.reciprocal_approx_fast(rec_bc[:], ps_den[:])
                        rec[l] = rec_bc
                    for l in heads:
                        nc.vector.tensor_mul(OT_sb[:, l, q0:q0 + 512],
                                             ps_ot[l][:], rec[l][:])
                    yield

            def stage_c(tbs, pools, b=b, OT_sb=OT_sb):
                """Generator: wo matmuls for the given token blocks.

                pools: PSUM pools to rotate ps_o through (pass the idle
                attention pools for the cross-batch tail so drain latency is
                fully hidden)."""
                npool = 0
                for nt in range(8):
                    wo_sb = wop.tile([128, 4, 512], BF, tag="wo",
                                     name="wo_sb")
                    nc.sync.dma_start(wo_sb[:], woR[:, nt])
                    for i, tb in enumerate(tbs):
                        pool, ptag = pools[npool % len(pools)]
                        npool += 1
                        ps_o = pool.tile([128, 512], F32, tag=ptag,
                                         name="ps_o")
                        for k in range(4):
                            nc.tensor.matmul(
                                ps_o[:], OT_sb[:, k, tb * 128:(tb + 1) * 128],
                                wo_sb[:, k, :], start=(k == 0), stop=(k == 3))
                        o_sb = outp.tile([128, 512], F32, tag="o",
                                         name="o_sb")
                        if tb % 2 == 0:
                            nc.scalar.copy(o_sb[:], ps_o[:])
                        else:
                            nc.vector.tensor_copy(o_sb[:], ps_o[:])
                        nc.sync.dma_start(
                            y[b * S + tb * 128:b * S + (tb + 1) * 128,
                              nt * 512:(nt + 1) * 512], o_sb[:])
                        if i % 2 == 1:
                            yield
                    yield

            # ---- batch schedule ----
            # stage A tt=0,1 zipped with the previous batch's stage-C tail
            def a_front():
                for u in stage_a(0, None if b == 0 else (b, 1)):
                    yield u
                for u in stage_a(1, (b, 2)):
                    yield u

            if pending_c_tail is not None:
                _zip_chunks(a_front(), pending_c_tail)
                pending_c_tail = None
            else:
                for _ in a_front():
                    pass
            # attention qh=0 zipped with stage A tt=2,3
            def a_tail():
                for u in stage_a(2, (b, 3)):
                    yield u
                nxt = (b + 1, 0) if b + 1 < B else None
                for u in stage_a(3, nxt):
                    yield u
            _zip_chunks(a_tail(), attn_half(0), lead=3)
            # attention qh=1 zipped with stage C for its ready token blocks
            _zip_chunks(stage_c([0, 1, 2, 3], [(psA, "psa")]), attn_half(1), lead=2)
            # the tb>=4 tail runs zipped into the next batch's stage A,
            # drawing PSUM from the attention pools (idle in that window)
            pending_c_tail = stage_c(
                [4, 5, 6, 7],
                [(psSC, "pssc"), (psOT, "psot")] + ([(psA, "psa")] if b == B - 1 else []))

        if pending_c_tail is not None:
            for _ in pending_c_tail:
                pass

    nc.compile()
    return nc


_CACHE = {}


def _get_program(causal):
    if causal not in _CACHE:
        _CACHE[causal] = _build_program(causal)
    return _CACHE[causal]


def kernel(x, wq_w, wq_a, wq_b, wk_w, wv_w, wv_a, wv_b, wo_w,
           freqs_cos, freqs_sin, mask, start_pos=0, _trace=False):
    assert int(np.asarray(start_pos)) == 0
    shared, cores, causal = _host_prep(
        x, wq_w, wq_a, wq_b, wk_w, wv_w, wv_a, wv_b, wo_w,
        freqs_cos, freqs_sin, mask)
    nc = _get_program(causal)
    in_maps = []
    for c in range(N_CORES):
        m = dict(xR=shared["xR"], cc=shared["cc"], ss=shared["ss"],
                 maskp=shared["maskp"])
        m.update(cores[c])
        in_maps.append(m)
    res = run_bass_kernel_spmd(nc, in_maps, list(range(N_CORES)),
                               trace=_trace)
    kernel._last_results = res
    acc = np.zeros((T, D), np.float32)
    for c in range(N_CORES):
        acc += np.asarray(res.results[c]["y"], np.float32)
    out = acc.reshape(B, S, D)
    return out.astype(np.asarray(x).dtype, copy=False)
